# revision 4
# baseline (speedup 1.0000x reference)
"""Trainium2 Bass kernel for nn_CustomAttn (conv1d -> channel self-attention -> conv1d -> 1x1).

Strategy: pure data-parallel over batch B=8 across 8 NeuronCores (one sample per core).

Per-core algorithm (C=512 channels, L=8192, channel-softmax attention):
  h  = relu(conv1d(x, w1))                       # S-path, fp32
  G  = h @ h.T            (Gram over L)          # replaces Q,K materialization:
  ST = Wk (G Wq^T)                               #   S = Q K^T = Wq G Wk^T
  AT = softmax(ST, over free dim)                # == softmax(S, axis=-2) transposed
  MT = (A Wv)^T                                  # fold Wv into A: y = A V = (A Wv) h
  y  = M h ; z = relu(y)                         # V-path, float32r (tf32-class)
  y2 = conv1d(z, w2) + b2                        # 5 shifted matmuls with halo'd z tiles
  out= w3 @ y2 + b3

G is symmetric: only upper-triangle chunk strips are accumulated; the lower
triangle is mirrored with PE transposes.

Precision: the softmax logits reach +-1e4 with top-2 gaps as small as 0.13, so the
S-path (conv1, Gram, S) must be fp32; the V-path tolerates float32r (11-bit mantissa).
"""
import sys
sys.path.insert(0, '/opt/trn_rl_repo')
from contextlib import ExitStack

import numpy as np

import concourse.bass as bass
import concourse.tile as tile
from concourse import bacc, mybir
from concourse.bass_utils import run_bass_kernel_spmd

F32 = mybir.dt.float32
F32R = mybir.dt.float32r
AX = mybir.AxisListType
AF = mybir.ActivationFunctionType

B, C_IN, C, KS, PAD = 8, 8, 512, 5, 2
FULL_L = 8192
P = 128
NCH = C // P          # 4 channel chunks
KIM = C_IN * KS + 1   # 41 im2col rows (+1 bias row)
NCORES = 8
LT = 512              # L tile (phase 3)
LC = 128              # l chunk (phase 1)
DCH = 1024            # xim DMA chunk width


def build_nc(L=FULL_L):
    nt = L // LT
    nlc = L // LC
    nc = bacc.Bacc("TRN2", target_bir_lowering=False, debug=False)

    xim_d = nc.dram_tensor("xim", [KIM, L], F32, kind="ExternalInput").ap()
    w1t_d = nc.dram_tensor("w1t", [KIM, C], F32, kind="ExternalInput").ap()
    ident_d = nc.dram_tensor("ident", [P, P], F32, kind="ExternalInput").ap()
    wqt_d = nc.dram_tensor("wqt", [C, C], F32, kind="ExternalInput").ap()
    wkt_d = nc.dram_tensor("wkt", [C, C], F32, kind="ExternalInput").ap()
    wv_d = nc.dram_tensor("wv", [C, C], F32R, kind="ExternalInput").ap()
    w2t_d = nc.dram_tensor("w2t", [KS, C, C], F32R, kind="ExternalInput").ap()
    w3t_d = nc.dram_tensor("w3t", [C, C_IN], F32R, kind="ExternalInput").ap()
    b2_d = nc.dram_tensor("b2", [C, 1], F32, kind="ExternalInput").ap()
    b3_d = nc.dram_tensor("b3", [C_IN, 1], F32, kind="ExternalInput").ap()
    out_d = nc.dram_tensor("out", [C_IN, L], F32, kind="ExternalOutput").ap()

    with tile.TileContext(nc) as tc, ExitStack() as top:
        wpool = top.enter_context(tc.tile_pool(name="weights", bufs=1))
        mtpool = top.enter_context(tc.tile_pool(name="mtp", bufs=1))
        h_pool = top.enter_context(tc.tile_pool(name="hsb", bufs=8))
        hps_pool = top.enter_context(tc.tile_pool(name="hps", bufs=2, space="PSUM"))

        # inputs phase 1 needs first: xim (chunked so DMA spreads across queues)
        xim_sb = wpool.tile([KIM, L], F32, tag="xim", name="xim")
        for i in range(0, L, DCH):
            e = min(L, i + DCH)
            nc.sync.dma_start(xim_sb[:, i:e], xim_d[:, i:e])
        w1t_sb = wpool.tile([KIM, C], F32, tag="w1t", name="w1t")
        nc.sync.dma_start(w1t_sb[:], w1t_d[:])
        ident_sb = wpool.tile([P, P], F32, tag="ident", name="ident")
        nc.sync.dma_start(ident_sb[:], ident_d[:])

        # f32r variants derived on-device (DVE rounds to f32r)
        ximr_sb = wpool.tile([KIM, L], F32R, tag="ximr", name="ximr")
        nc.vector.tensor_copy(ximr_sb[:], xim_sb[:])
        w1tr_sb = wpool.tile([KIM, C], F32R, tag="w1tr", name="w1tr")
        nc.vector.tensor_copy(w1tr_sb[:], w1t_sb[:])

        w3t_sb = [wpool.tile([P, C_IN], F32R, tag=f"w3t{ci}", name=f"w3t{ci}")
                  for ci in range(NCH)]
        for ci in range(NCH):
            nc.sync.dma_start(w3t_sb[ci][:], w3t_d[ci * P:(ci + 1) * P, :])
        b2_sb = [wpool.tile([P, 1], F32, tag=f"b2_{ci}", name=f"b2_{ci}")
                 for ci in range(NCH)]
        for ci in range(NCH):
            nc.sync.dma_start(b2_sb[ci][:], b2_d[ci * P:(ci + 1) * P, :])
        b3_sb = wpool.tile([C_IN, 1], F32, tag="b3", name="b3")
        nc.sync.dma_start(b3_sb[:], b3_d[:])

        mt_sb = [mtpool.tile([P, C], F32R, tag=f"mt{i}", name=f"mt{i}")
                 for i in range(NCH)]

        h_tiles = {}

        def compute_h(t):
            hcs = []
            for ci in range(NCH):
                h_ps = hps_pool.tile([P, LT], F32, tag="hps", name="hps")
                nc.tensor.matmul(h_ps[:], w1tr_sb[:, ci * P:(ci + 1) * P],
                                 ximr_sb[:, t * LT:(t + 1) * LT],
                                 start=True, stop=True)
                h_sb = h_pool.tile([P, LT], F32R, tag="hsb", name="hsb")
                nc.scalar.activation(h_sb[:], h_ps[:], AF.Relu)
                hcs.append(h_sb)
            h_tiles[t] = hcs

        # ---------------- Phase 1+2: attention matrix ----------------
        with ExitStack() as ph12:
            qkv = ph12.enter_context(tc.tile_pool(name="qkv", bufs=1))
            wqt_sb = [qkv.tile([P, C], F32, tag=f"wqt{i}", name=f"wqt{i}")
                      for i in range(NCH)]
            wkt_sb = [qkv.tile([P, C], F32, tag=f"wkt{i}", name=f"wkt{i}")
                      for i in range(NCH)]
            wv_sb = [qkv.tile([P, C], F32R, tag=f"wv{i}", name=f"wv{i}")
                     for i in range(NCH)]
            for i in range(NCH):
                nc.sync.dma_start(wqt_sb[i][:], wqt_d[i * P:(i + 1) * P, :])
                nc.sync.dma_start(wkt_sb[i][:], wkt_d[i * P:(i + 1) * P, :])
                nc.sync.dma_start(wv_sb[i][:], wv_d[i * P:(i + 1) * P, :])

            gsb_pool = ph12.enter_context(tc.tile_pool(name="gsb", bufs=1))
            g_sb = [gsb_pool.tile([P, C], F32, tag=f"gsb{i}", name=f"gsb{i}")
                    for i in range(NCH)]

            with ExitStack() as ph1:
                gps_pool = ph1.enter_context(
                    tc.tile_pool(name="gps", bufs=1, space="PSUM"))
                htps_pool = ph1.enter_context(
                    tc.tile_pool(name="htps", bufs=2, space="PSUM"))
                ht_pool = ph1.enter_context(tc.tile_pool(name="ht", bufs=3))

                g_ps = [gps_pool.tile([P, C], F32, tag=f"g{i}", name=f"g{i}")
                        for i in range(NCH)]
                ht_sb = [None] * nlc

                def emit_ht(lc):
                    ht_ps = htps_pool.tile([LC, C], F32, tag="htps", name="htps")
                    nc.tensor.matmul(ht_ps[:], xim_sb[:, lc * LC:(lc + 1) * LC],
                                     w1t_sb[:], start=True, stop=True)
                    ht_sb[lc] = ht_pool.tile([LC, C], F32, tag="htsb", name="htsb")
                    nc.scalar.activation(ht_sb[lc][:], ht_ps[:], AF.Relu)

                emit_ht(0)
                if nlc > 1:
                    emit_ht(1)
                for lc in range(nlc):
                    if lc + 2 < nlc:
                        emit_ht(lc + 2)
                    # upper-triangle strips of the symmetric Gram matrix
                    for ci in range(NCH):
                        nc.tensor.matmul(g_ps[ci][:, ci * P:],
                                         ht_sb[lc][:, ci * P:(ci + 1) * P],
                                         ht_sb[lc][:, ci * P:],
                                         start=(lc == 0), stop=(lc == nlc - 1))
                    ht_sb[lc] = None

                for i in range(NCH):
                    nc.vector.tensor_copy(g_sb[i][:, i * P:], g_ps[i][:, i * P:])

            # mirror the lower triangle: g_sb[cj][:, ci] = g_sb[ci][:, cj].T
            with ExitStack() as phm:
                tp_pool = phm.enter_context(
                    tc.tile_pool(name="tpps", bufs=2, space="PSUM"))
                for ci in range(NCH):
                    for cj in range(ci + 1, NCH):
                        tp = tp_pool.tile([P, P], F32, tag="tp", name="tp")
                        nc.tensor.transpose(tp[:], g_sb[ci][:, cj * P:(cj + 1) * P],
                                            ident_sb[:])
                        nc.vector.tensor_copy(g_sb[cj][:, ci * P:(ci + 1) * P], tp[:])

            # Phase 2: U = G Wq^T ; ST = Wk U ; softmax ; MT = (A Wv)^T
            with ExitStack() as ph2:
                p2ps = ph2.enter_context(tc.tile_pool(name="p2ps", bufs=3, space="PSUM"))
                p2sb = ph2.enter_context(tc.tile_pool(name="p2sb", bufs=1))
                p2tmp = ph2.enter_context(tc.tile_pool(name="p2tmp", bufs=2))

                u_sb = []
                for i in range(NCH):
                    u_ps = p2ps.tile([P, C], F32, tag="p2", name="p2")
                    for j in range(NCH):
                        nc.tensor.matmul(u_ps[:], g_sb[j][:, i * P:(i + 1) * P],
                                         wqt_sb[j][:],
                                         start=(j == 0), stop=(j == NCH - 1))
                    t = p2sb.tile([P, C], F32, tag=f"usb{i}", name=f"usb{i}")
                    nc.vector.tensor_copy(t[:], u_ps[:])
                    u_sb.append(t)

                at_sb = []
                for i in range(NCH):
                    st_ps = p2ps.tile([P, C], F32, tag="p2", name="p2")
                    for j in range(NCH):
                        nc.tensor.matmul(st_ps[:], wkt_sb[j][:, i * P:(i + 1) * P],
                                         u_sb[j][:],
                                         start=(j == 0), stop=(j == NCH - 1))
                    m = p2tmp.tile([P, 1], F32, tag="m", name="m")
                    nc.vector.reduce_max(m[:], st_ps[:], axis=AX.X)
                    nm = p2tmp.tile([P, 1], F32, tag="nm", name="nm")
                    nc.vector.tensor_scalar_mul(nm[:], m[:], -1.0)
                    e = p2tmp.tile([P, C], F32, tag="e", name="e")
                    ssum = p2tmp.tile([P, 1], F32, tag="ssum", name="ssum")
                    nc.scalar.activation(e[:], st_ps[:], AF.Exp, bias=nm[:],
                                         accum_out=ssum[:])
                    r = p2tmp.tile([P, 1], F32, tag="r", name="r")
                    nc.vector.reciprocal(r[:], ssum[:])
                    a = p2sb.tile([P, C], F32R, tag=f"at{i}", name=f"at{i}")
                    nc.vector.tensor_scalar_mul(a[:], e[:], r[:])
                    at_sb.append(a)

                # keep the PE busy during the softmax: prefetch phase-3 h tiles
                compute_h(0)
                if nt > 1:
                    compute_h(1)

                for i in range(NCH):
                    mt_ps = p2ps.tile([P, C], F32, tag="p2", name="p2")
                    for j in range(NCH):
                        nc.tensor.matmul(mt_ps[:], wv_sb[j][:, i * P:(i + 1) * P],
                                         at_sb[j][:],
                                         start=(j == 0), stop=(j == NCH - 1))
                    nc.vector.tensor_copy(mt_sb[i][:], mt_ps[:])

        # ---------------- Phase 3: y = M h ; z = relu(y) ; conv2 ; conv3 ----------------
        with ExitStack() as ph3:
            w2pool = ph3.enter_context(tc.tile_pool(name="w2pool", bufs=1))
            w2t_sb = [[w2pool.tile([P, C], F32R, tag=f"w2t{k}_{ci}", name=f"w2t{k}_{ci}")
                       for ci in range(NCH)] for k in range(KS)]
            for k in range(KS):
                for ci in range(NCH):
                    nc.sync.dma_start(w2t_sb[k][ci][:], w2t_d[k, ci * P:(ci + 1) * P, :])

            yps_pool = ph3.enter_context(tc.tile_pool(name="yps", bufs=2, space="PSUM"))
            y2ps_pool = ph3.enter_context(tc.tile_pool(name="y2ps", bufs=2, space="PSUM"))
            ops_pool = ph3.enter_context(tc.tile_pool(name="ops", bufs=1, space="PSUM"))
            z_pool = ph3.enter_context(tc.tile_pool(name="zsb", bufs=4))
            y2_pool = ph3.enter_context(tc.tile_pool(name="y2sb", bufs=5))
            o_pool = ph3.enter_context(tc.tile_pool(name="osb", bufs=2))

            z_tiles = [None] * nt

            def compute_y_z(t):
                hcs = h_tiles.pop(t)
                zt = z_pool.tile([P, NCH, LT + 4], F32R, tag="z", name="z")
                for ci in range(NCH):
                    y_ps = yps_pool.tile([P, LT], F32, tag="yps", name="yps")
                    for cj in range(NCH):
                        nc.tensor.matmul(y_ps[:], mt_sb[cj][:, ci * P:(ci + 1) * P],
                                         hcs[cj][:],
                                         start=(cj == 0), stop=(cj == NCH - 1))
                    nc.scalar.activation(zt[:, ci, 2:2 + LT], y_ps[:], AF.Relu)
                if t == 0:
                    nc.vector.memset(zt[:, :, 0:2].bitcast(F32), 0.0)
                else:
                    nc.vector.tensor_copy(zt[:, :, 0:2], z_tiles[t - 1][:, :, LT:LT + 2])
                    nc.vector.tensor_copy(z_tiles[t - 1][:, :, LT + 2:LT + 4],
                                          zt[:, :, 2:4])
                z_tiles[t] = zt

            def emit_out(t):
                zt = z_tiles[t]
                y2cs = []
                for oi in range(NCH):
                    y2_ps = y2ps_pool.tile([P, LT], F32, tag="y2ps", name="y2ps")
                    first = True
                    for k in range(KS):
                        for ci in range(NCH):
                            nc.tensor.matmul(y2_ps[:],
                                             w2t_sb[k][ci][:, oi * P:(oi + 1) * P],
                                             zt[:, ci, k:k + LT],
                                             start=first,
                                             stop=(k == KS - 1 and ci == NCH - 1))
                            first = False
                    y2_sb = y2_pool.tile([P, LT], F32R, tag="y2sb", name="y2sb")
                    nc.scalar.activation(y2_sb[:], y2_ps[:], AF.Identity,
                                         bias=b2_sb[oi][:])
                    y2cs.append(y2_sb)
                o_ps = ops_pool.tile([C_IN, LT], F32, tag="ops", name="ops")
                for ci in range(NCH):
                    nc.tensor.matmul(o_ps[:], w3t_sb[ci][:], y2cs[ci][:],
                                     start=(ci == 0), stop=(ci == NCH - 1))
                o_sb = o_pool.tile([C_IN, LT], F32, tag="osb", name="osb")
                nc.scalar.activation(o_sb[:], o_ps[:], AF.Identity, bias=b3_sb[:])
                nc.sync.dma_start(out_d[:, t * LT:(t + 1) * LT], o_sb[:])
                z_tiles[t] = None

            compute_y_z(0)
            if nt > 1:
                if nt > 2:
                    compute_h(2)
                compute_y_z(1)
            for t in range(2, nt):
                if t + 1 < nt:
                    compute_h(t + 1)
                emit_out(t - 2)
                compute_y_z(t)
            nc.vector.memset(z_tiles[nt - 1][:, :, LT + 2:LT + 4].bitcast(F32), 0.0)
            if nt > 1:
                emit_out(nt - 2)
            emit_out(nt - 1)

    nc.compile()
    return nc


def prep_host(inputs, L=FULL_L):
    """Host-side input prep: im2col of x, transposed weight layouts."""
    x = np.ascontiguousarray(np.asarray(inputs["x"], np.float32))
    w1 = np.asarray(inputs["w_conv1"], np.float32)
    b1 = np.asarray(inputs["b_conv1"], np.float32)
    nb = x.shape[0]

    xim = np.zeros((nb, KIM, L), np.float32)
    for k in range(KS):
        sh = k - PAD
        lo, hi = max(0, -sh), min(L, L - sh)
        xim[:, k * C_IN:(k + 1) * C_IN, lo:hi] = x[:, :, lo + sh:hi + sh]
    xim[:, KIM - 1, :] = 1.0

    w1t = np.zeros((KIM, C), np.float32)
    w1t[:KIM - 1] = w1.transpose(2, 1, 0).reshape(KIM - 1, C)
    w1t[KIM - 1] = b1

    shared = {
        "w1t": w1t,
        "ident": np.eye(P, dtype=np.float32),
        "wqt": np.ascontiguousarray(np.asarray(inputs["w_q"], np.float32).T),
        "wkt": np.ascontiguousarray(np.asarray(inputs["w_k"], np.float32).T),
        "wv": np.ascontiguousarray(np.asarray(inputs["w_v"], np.float32)),
        "w2t": np.ascontiguousarray(
            np.asarray(inputs["w_conv2"], np.float32).transpose(2, 1, 0)),
        "w3t": np.ascontiguousarray(np.asarray(inputs["w_conv3"], np.float32).T),
        "b2": np.asarray(inputs["b_conv2"], np.float32).reshape(C, 1),
        "b3": np.asarray(inputs["b_conv3"], np.float32).reshape(C_IN, 1),
    }
    in_maps = []
    for i in range(nb):
        m = dict(shared)
        m["xim"] = np.ascontiguousarray(xim[i])
        in_maps.append(m)
    return in_maps


_NC_CACHE = {}


def _get_nc(L=FULL_L):
    if L not in _NC_CACHE:
        _NC_CACHE[L] = build_nc(L)
    return _NC_CACHE[L]


def kernel(**inputs) -> np.ndarray:
    nc = _get_nc(FULL_L)
    in_maps = prep_host(inputs, FULL_L)
    res = run_bass_kernel_spmd(nc, in_maps, list(range(NCORES)))
    out = np.stack([res.results[i]["out"] for i in range(NCORES)], axis=0)
    return out.astype(np.float32)


if __name__ == "__main__":
    rng = np.random.default_rng(0)
    fake = {
        "x": rng.standard_normal((B, C_IN, FULL_L), dtype=np.float32),
        "w_conv1": rng.standard_normal((C, C_IN, KS), dtype=np.float32) / np.sqrt(C_IN * KS),
        "b_conv1": np.zeros(C, np.float32),
        "w_q": rng.standard_normal((C, C), dtype=np.float32) / np.sqrt(C),
        "w_k": rng.standard_normal((C, C), dtype=np.float32) / np.sqrt(C),
        "w_v": rng.standard_normal((C, C), dtype=np.float32) / np.sqrt(C),
        "w_conv2": rng.standard_normal((C, C, KS), dtype=np.float32) / np.sqrt(C * KS),
        "b_conv2": np.zeros(C, np.float32),
        "w_conv3": rng.standard_normal((C_IN, C), dtype=np.float32) / np.sqrt(C),
        "b_conv3": np.zeros(C_IN, np.float32),
    }
    out = kernel(**fake)
    print("kernel out:", out.shape, out.dtype)
